# revision 13
# baseline (speedup 1.0000x reference)
"""MoE gate Trainium2 kernel, v2: natural-layout DMA + on-chip PE transposes +
weight-stationary fp32 GEMM.

Per core (2048 tokens):
  - W is passed host-transposed [7168,256] (small replicated tensor, same
    spirit as the host bias broadcast) and loaded as 56 wT tiles [128k,256e]
    with contiguous-row DMAs.
  - Tokens processed in 2 half-blocks of 1024. Per half, per K-chunk of 14
    K-tiles: natural h loads [128t, 1792], 8 PE transposes per K-tile into
    PSUM (packed 4 per bank), DVE evacuation into hT [128k, 1024t], then 4
    weight-stationary matmuls [128e,512t] accumulating over all 56 K-tiles.
  - Output logits^T [2*128e, 1024t] transposed back on PE to [128t, 256e],
    sigmoid + top-k routing on ACT/DVE (max8 / max_index / match_replace).
"""

import numpy as np

import concourse.bass as bass
import concourse.mybir as mybir
import concourse.tile as tile
from concourse import bacc
from concourse.bass_utils import run_bass_kernel_spmd
from concourse.masks import make_identity

N_CORES = 8
T_FULL = 16384
H = 7168
E = 256
TOP_K = 8
N_GROUP = 8
PER_GROUP = E // N_GROUP
ROUTED_SCALING = 2.5

T_CORE = T_FULL // N_CORES  # 2048
KT = H // 128  # 56
HALF = 1024
N_HALF = T_CORE // HALF  # 2
TT_HALF = HALF // 128  # 8
KC = 4  # K chunks
KPC = KT // KC  # 14 K-tiles per chunk

F32 = mybir.dt.float32
U32 = mybir.dt.uint32
I32 = mybir.dt.int32
BIG = 1.0e9

LAST_EXEC_NS = None


def _chain(prev, cur):
    if prev is not None:
        bass._add_dep_helper(cur.ins, prev.ins, sync=False, reason="order")
    return cur


def build_nc(repeat=1):
    nc = bacc.Bacc(None)
    h_ext = nc.declare_dram_parameter("h", [T_CORE, H], F32, isOutput=False)
    w_ext = nc.declare_dram_parameter("w", [H, E], F32, isOutput=False)
    b_ext = nc.declare_dram_parameter("b", [128, E], F32, isOutput=False)
    idx_ext = nc.declare_dram_parameter("idx", [T_CORE, TOP_K], I32, isOutput=True)
    wout_ext = nc.declare_dram_parameter("wout", [T_CORE, TOP_K], F32, isOutput=True)

    # natural views: rows contiguous in DRAM
    h_nat = h_ext[:].rearrange("(tt p) k -> tt p k", p=128)  # [16,128,7168]
    wT_nat = w_ext[:].rearrange("(kt p) e -> kt p e", p=128)  # [56,128,256]

    with tile.TileContext(nc) as tc:
        with (
            tc.tile_pool(name="wpool", bufs=1) as wpool,
            tc.tile_pool(name="hnat", bufs=10) as hnat_pool,
            tc.tile_pool(name="hT", bufs=3) as hT_pool,
            tc.tile_pool(name="lgT", bufs=2) as lgT_pool,
            tc.tile_pool(name="route", bufs=2) as route_pool,
            tc.tile_pool(name="small", bufs=2) as small_pool,
            tc.tile_pool(name="pst", bufs=4, space="PSUM") as pst_pool,
            tc.tile_pool(name="psg", bufs=4, space="PSUM") as psg_pool,
        ):
            ident = wpool.tile([128, 128], F32, tag="ident")
            make_identity(nc, ident[:])

            bias_sb = wpool.tile([128, E], F32, tag="bias")
            nc.sync.dma_start(out=bias_sb[:], in_=b_ext[:])

            # ---- W arrives host-transposed [H, E]: direct natural loads
            wT = [
                wpool.tile([128, E], F32, tag=f"wT{k}", name=f"wT{k}")
                for k in range(KT)
            ]
            for k in range(KT):
                nc.sync.dma_start(out=wT[k][:], in_=wT_nat[k])
            tr_prev = None

            # ---- main loop
            prev_stop = {}  # (e, c) -> last mm of previous half (psum slot chain)
            prev_tr = tr_prev  # PE transpose chain across banks
            for rep in range(repeat):
                for half in range(N_HALF):
                    t0 = half * TT_HALF  # first token-tile of half
                    gps = {}
                    for e in range(2):
                        for c in range(2):
                            gps[(e, c)] = psg_pool.tile(
                                [128, 512], F32, tag="psg",
                                name=f"g{rep}_{half}_{e}_{c}",
                            )
                    first_mm = {}
                    for kc in range(KC):
                        hn = []
                        for tt in range(TT_HALF):
                            t = t0 + tt
                            hh = hnat_pool.tile(
                                [128, KPC * 128], F32, tag="h_nat"
                            )
                            nc.sync.dma_start(
                                out=hh[:],
                                in_=h_nat[t][:, kc * KPC * 128 : (kc + 1) * KPC * 128],
                            )
                            hn.append(hh)
                        for kk in range(KPC):
                            k = kc * KPC + kk
                            hT = hT_pool.tile([128, HALF], F32, tag="hT")
                            for pair in range(2):  # 4 transposes per bank
                                pst = pst_pool.tile(
                                    [128, 512], F32, tag="pst",
                                    name=f"p{rep}_{half}_{k}_{pair}",
                                )
                                for j in range(4):
                                    tt = pair * 4 + j
                                    tr = nc.tensor.matmul(
                                        pst[:, j * 128 : (j + 1) * 128],
                                        hn[tt][:, kk * 128 : (kk + 1) * 128],
                                        ident[:],
                                        is_transpose=True,
                                        start=(j == 0),
                                        stop=(j == 3),
                                    )
                                    prev_tr = _chain(prev_tr, tr)
                                nc.vector.tensor_copy(
                                    hT[:, pair * 512 : (pair + 1) * 512], pst[:]
                                )
                            for e in range(2):
                                for c in range(2):
                                    mm = nc.tensor.matmul(
                                        gps[(e, c)][:],
                                        wT[k][:, e * 128 : (e + 1) * 128],
                                        hT[:, c * 512 : (c + 1) * 512],
                                        start=(k == 0),
                                        stop=(k == KT - 1),
                                    )
                                    if k == 0:
                                        first_mm[(e, c)] = mm
                                        if (e, c) in prev_stop:
                                            _chain(prev_stop[(e, c)], mm)
                                    if k == KT - 1:
                                        prev_stop[(e, c)] = mm

                    # ---- logits^T evacuation + output transposes
                    lgT = []
                    for e in range(2):
                        lg = lgT_pool.tile([128, HALF], F32, tag="lgT")
                        for c in range(2):
                            nc.vector.tensor_copy(
                                lg[:, c * 512 : (c + 1) * 512], gps[(e, c)][:]
                            )
                        lgT.append(lg)

                    for tp in range(TT_HALF // 2):  # 2 t-tiles per bank
                        pst = pst_pool.tile(
                            [128, 512], F32, tag="pst",
                            name=f"o{rep}_{half}_{tp}",
                        )
                        for j in range(4):
                            tt = tp * 2 + j // 2
                            e = j % 2
                            tr = nc.tensor.matmul(
                                pst[:, j * 128 : (j + 1) * 128],
                                lgT[e][:, tt * 128 : (tt + 1) * 128],
                                ident[:],
                                is_transpose=True,
                                start=(j == 0),
                                stop=(j == 3),
                            )
                            prev_tr = _chain(prev_tr, tr)
                        for j in range(2):
                            t = t0 + tp * 2 + j
                            _routing(
                                nc, tc, route_pool, small_pool,
                                pst[:, j * 256 : (j + 1) * 256],
                                bias_sb, idx_ext, wout_ext, t,
                            )

    nc.finalize()
    return nc


def _routing(nc, tc, route_pool, small_pool, logits_ap, bias_sb, idx_ext,
             wout_ext, t):
    sc = route_pool.tile([128, E], F32, tag="sc")
    nc.scalar.activation(sc[:], logits_ap, mybir.ActivationFunctionType.Sigmoid)
    scb = route_pool.tile([128, E], F32, tag="scb")
    nc.vector.tensor_add(scb[:], sc[:], bias_sb[:])

    gmax = small_pool.tile([128, N_GROUP * 8], F32, tag="gmax")
    for g in range(N_GROUP):
        nc.vector.max(
            gmax[:, g * 8 : g * 8 + 8],
            scb[:, g * PER_GROUP : (g + 1) * PER_GROUP],
        )
    gs = small_pool.tile([128, N_GROUP], F32, tag="gs")
    gm3 = gmax[:].rearrange("p (g k) -> p g k", k=8)
    nc.vector.tensor_add(gs[:], gm3[:, :, 0], gm3[:, :, 1])

    g8 = small_pool.tile([128, 8], F32, tag="g8")
    nc.vector.max(g8[:], gs[:])
    gpen = small_pool.tile([128, N_GROUP], F32, tag="gpen")
    nc.vector.tensor_scalar(
        gpen[:], gs[:], g8[:, 3:4], -1.0,
        mybir.AluOpType.is_ge, mybir.AluOpType.add,
    )
    tmp = route_pool.tile([128, E], F32, tag="tmp")
    tmp3 = tmp[:].rearrange("p (g e) -> p g e", e=PER_GROUP)
    scb3 = scb[:].rearrange("p (g e) -> p g e", e=PER_GROUP)
    gpen3 = gpen[:, :, None].to_broadcast([128, N_GROUP, PER_GROUP])
    nc.vector.scalar_tensor_tensor(
        tmp3, gpen3, BIG, scb3, mybir.AluOpType.mult, mybir.AluOpType.add,
    )

    v8 = small_pool.tile([128, 8], F32, tag="v8")
    idx8 = small_pool.tile([128, 8], U32, tag="idx8")
    nc.vector.max(v8[:], tmp[:])
    nc.vector.max_index(idx8[:], v8[:], tmp[:])

    mr = route_pool.tile([128, E], F32, tag="mr")
    nc.vector.match_replace(mr[:], v8[:], tmp[:], 2.0 * BIG)
    m01 = route_pool.tile([128, E], F32, tag="m01")
    nc.vector.tensor_scalar(
        m01[:], mr[:], 1.5 * BIG, None, mybir.AluOpType.is_ge
    )
    ssel = route_pool.tile([128, E], F32, tag="ssel")
    nc.vector.tensor_mul(ssel[:], sc[:], m01[:])

    s8 = small_pool.tile([128, 8], F32, tag="s8")
    i8 = small_pool.tile([128, 8], U32, tag="i8")
    nc.vector.max(s8[:], ssel[:])
    nc.vector.max_index(i8[:], s8[:], ssel[:])

    idx8f = small_pool.tile([128, 8], F32, tag="idx8f")
    i8f = small_pool.tile([128, 8], F32, tag="i8f")
    nc.vector.tensor_copy(idx8f[:], idx8[:])
    nc.vector.tensor_copy(i8f[:], i8[:])
    iseq = small_pool.tile([128, 64], F32, tag="iseq")
    iseq3 = iseq[:].rearrange("p (j m) -> p j m", m=8)
    nc.vector.tensor_tensor(
        iseq3,
        idx8f[:, :, None].to_broadcast([128, 8, 8]),
        i8f[:, None, :].to_broadcast([128, 8, 8]),
        mybir.AluOpType.is_equal,
    )
    wsel = small_pool.tile([128, 64], F32, tag="wsel")
    wsel3 = wsel[:].rearrange("p (j m) -> p j m", m=8)
    nc.vector.tensor_tensor(
        wsel3, iseq3, s8[:, None, :].to_broadcast([128, 8, 8]),
        mybir.AluOpType.mult,
    )
    wj = small_pool.tile([128, 8], F32, tag="wj")
    nc.vector.reduce_sum(wj[:], wsel3, axis=mybir.AxisListType.X)

    sum8 = small_pool.tile([128, 1], F32, tag="sum8")
    nc.vector.reduce_sum(sum8[:], wj[:], axis=mybir.AxisListType.X)
    seps = small_pool.tile([128, 1], F32, tag="seps")
    nc.vector.tensor_scalar_add(seps[:], sum8[:], 1.0e-20)
    rec = small_pool.tile([128, 1], F32, tag="rec")
    nc.vector.reciprocal(rec[:], seps[:])
    wout = small_pool.tile([128, 8], F32, tag="wout")
    nc.vector.tensor_scalar(
        wout[:], wj[:], rec[:, 0:1], ROUTED_SCALING,
        mybir.AluOpType.mult, mybir.AluOpType.mult,
    )

    nc.sync.dma_start(
        out=idx_ext[t * 128 : (t + 1) * 128, :], in_=idx8[:].bitcast(I32)
    )
    nc.sync.dma_start(
        out=wout_ext[t * 128 : (t + 1) * 128, :], in_=wout[:]
    )


_NC_CACHE = None


def kernel(hidden_states, weight, e_score_correction_bias):
    global _NC_CACHE, LAST_EXEC_NS
    h = np.ascontiguousarray(
        np.asarray(hidden_states, dtype=np.float32)
    ).reshape(T_FULL, H)
    w = np.ascontiguousarray(np.asarray(weight, dtype=np.float32).T)
    b = np.asarray(e_score_correction_bias, dtype=np.float32)
    b_bcast = np.ascontiguousarray(np.broadcast_to(b[None, :], (128, E)))

    if _NC_CACHE is None:
        _NC_CACHE = build_nc()
    nc = _NC_CACHE

    in_maps = [
        {"h": h[c * T_CORE : (c + 1) * T_CORE], "w": w, "b": b_bcast}
        for c in range(N_CORES)
    ]
    res = run_bass_kernel_spmd(nc, in_maps, core_ids=list(range(N_CORES)))
    LAST_EXEC_NS = res.exec_time_ns

    idx = np.concatenate([res.results[c]["idx"] for c in range(N_CORES)], axis=0)
    wout = np.concatenate([res.results[c]["wout"] for c in range(N_CORES)], axis=0)
    return idx.astype(np.int32), wout.astype(np.float32)
